# revision 23
# baseline (speedup 1.0000x reference)
"""Trainium2 Bass kernel for retrieval_knn (nn_CLI_v1_63702954934484).

Reference computation (per batch b):
    dist[n,m] = ||ca[n] - cb[m]|| / 128                         [Na, Nb]
    idx       = argtop4-smallest(dist[n,:])                     [Na, 4]
    dw        = R - clip(dist_top4, 0, R)                       [Na, 4]
    h         = [b_f, a_f - b_f]  (b_f = feats_b[idx])          [Na, 4, 2D]
    fused     = sum_k relu(h @ W + bias) * dw                   [Na, D]
    out       = [feats_a, fused]                                [Na, 2D]

v3: single flat software pipeline per core (2 batches), emission-ordered
so every engine runs back-to-back:
  * PE: 64 "rounds", each = 1 distance half-tile (2x512-col fp16 matmuls,
    packed -(dist2+m/2048) values) + 1 feature tile (Yb or Ya, 4 matmuls).
    Continuous occupancy keeps the PE at the 2.4GHz p-state (it drops to
    1.2GHz whenever the engine idles and needs ~3us busy to re-ramit).
  * DVE: max8 scans trail the dist matmuls; z-adds and k-sums for the MLP
    combine are interleaved between later scans.
  * Act: PSUM->SBUF fp16 copies of Ya/Yb + relu(z)*dw (activation scale).
  * Pool: Q7 dma_gather of Yb rows (2 tiles / 1024 idx per call), round-
    robined over 4 SWDGE queues so descriptor rings never force a drain.
  * top-4 via max8 on packed fp32 PSUM: value = -(dist2 + m/2048) gives
    bit-exact jax.lax.top_k ordering incl. index tie-breaks; indices are
    unpacked arithmetically (no max_index rescan).

Sharding: data-parallel over batch (16 batches -> 8 cores x 2).
"""

import sys

sys.path.insert(0, "/opt/trn_rl_repo")

import numpy as np

import concourse.bass as bass
import concourse.mybir as mybir
import concourse.tile as tile
from concourse import bacc
from concourse.library_config import mlp as mlp_lib

F32 = mybir.dt.float32
F16 = mybir.dt.float16
U32 = mybir.dt.uint32
I16 = mybir.dt.int16

P = 128          # partitions
D = 512          # feature dim
KNN = 4          # neighbors
R = 0.5
FULL_SCALE = 128.0

B = 16           # full batch
N_CORES = 8
BLOC = B // N_CORES  # batches per core

NA = 2048
NB = 2048
NT = NA // P     # n-tiles per batch
DT = D // P      # 128-chunks of the feature dim
HALF = 1024      # distance column chunk (2 PSUM banks)

AF = mybir.ActivationFunctionType
ALU = mybir.AluOpType

N_QUEUES = 4     # SWDGE queues for dma_gather round-robin
GW = 2           # tiles per dma_gather call

# idx-build groups per batch: (start_tile, num_tiles). First b0 group is
# small so the first gather can be issued early (Pool is the critical
# engine; it must start as soon as possible).
GROUPS = {0: [(0, 8), (8, 8)], 1: [(0, 8), (8, 8)]}


def build_bass(bloc=BLOC, na=NA, nb=NB, with_bias=False):
    nc = bacc.Bacc("TRN2", debug=False, num_swdge_queues=N_QUEUES)
    fatT = nc.dram_tensor("fatT", [bloc, DT, P, na], F16, kind="ExternalInput").ap()
    fbtT = nc.dram_tensor("fbtT", [bloc, DT, P, nb], F16, kind="ExternalInput").ap()
    phia = nc.dram_tensor("phia", [bloc, 8, na], F16, kind="ExternalInput").ap()
    phib = nc.dram_tensor("phib", [bloc, 8, nb], F16, kind="ExternalInput").ap()
    w2 = nc.dram_tensor("w2", [DT, P, D], F16, kind="ExternalInput").ap()
    wd = nc.dram_tensor("wd", [DT, P, D], F16, kind="ExternalInput").ap()
    biasw = nc.dram_tensor("biasw", [1, D], F16, kind="ExternalInput").ap()
    ident = nc.dram_tensor("ident", [P, P], F16, kind="ExternalInput").ap()
    rep = nc.dram_tensor("rep", [16, P], F16, kind="ExternalInput").ap()
    out = nc.dram_tensor("out", [bloc, na, D], F16, kind="ExternalOutput").ap()

    nc.gpsimd.load_library(mlp_lib)
    with tile.TileContext(nc) as tc:
        _kern(tc, fatT, fbtT, phia, phib, w2, wd, biasw, ident, rep, out,
              bloc=bloc, na=na, nb=nb, with_bias=with_bias)
    nc.compile()
    return nc


def _kern(tc, fatT, fbtT, phia, phib, w2, wd, biasw, ident, rep, out, *,
          bloc, na, nb, with_bias):
    nc = tc.nc
    nt = na // P

    with (
        tc.tile_pool(name="const", bufs=1) as cpool,
        tc.tile_pool(name="wpool", bufs=1) as wpool,
        tc.tile_pool(name="feat", bufs=1) as fpool,
        tc.tile_pool(name="phi", bufs=1) as phipool,
        tc.tile_pool(name="tk", bufs=1) as tkpool,
        tc.tile_pool(name="ext", bufs=2) as epool,
        tc.tile_pool(name="h16p", bufs=3) as hpool,
        tc.tile_pool(name="io", bufs=3) as iopool,
        tc.tile_pool(name="ya", bufs=1) as yapool,
        tc.tile_pool(name="gat", bufs=3) as gpool,
        tc.tile_pool(name="mlp", bufs=2) as mpool,
        tc.tile_pool(name="fusedp", bufs=10) as fpool2,
        tc.tile_pool(name="dram", bufs=1, space="DRAM") as dpool,
        tc.tile_pool(name="dist_ps", bufs=2, space="PSUM") as dps,
        tc.tile_pool(name="mm_ps", bufs=3, space="PSUM") as mmps,
        tc.tile_pool(name="tp_ps", bufs=1, space="PSUM") as tpps,
    ):
        rconst = cpool.tile([P, 1], F32, name="rconst")
        nc.vector.memset(rconst, R)
        ones_t = cpool.tile([1, P], F16, name="ones_t")
        nc.vector.memset(ones_t, 1.0)

        # resident weights (fp16)
        w2_sb = wpool.tile([P, DT, D], F16, name="w2_sb")
        wd_sb = wpool.tile([P, DT, D], F16, name="wd_sb")
        for j in range(DT):
            nc.sync.dma_start(out=w2_sb[:, j, :], in_=w2[j])
            nc.sync.dma_start(out=wd_sb[:, j, :], in_=wd[j])
        bias_sb = wpool.tile([1, D], F16, name="bias_sb")
        if with_bias:
            nc.sync.dma_start(out=bias_sb, in_=biasw)
        ident_sb = wpool.tile([P, P], F16, name="ident_sb")
        rep_sb = wpool.tile([16, P], F16, name="rep_sb")
        nc.sync.dma_start(out=ident_sb, in_=ident)
        nc.sync.dma_start(out=rep_sb, in_=rep)

        # ---- upfront loads for both batches ----
        phia_sb, phib_sb, fat_sb, fbt_sb = {}, {}, {}, {}
        for b in range(bloc):
            phia_sb[b] = phipool.tile([8, na], F16, tag=f"phia{b}",
                                      name=f"phia_sb{b}")
            phib_sb[b] = phipool.tile([8, nb], F16, tag=f"phib{b}",
                                      name=f"phib_sb{b}")
            nc.sync.dma_start(out=phia_sb[b], in_=phia[b])
            nc.sync.dma_start(out=phib_sb[b], in_=phib[b])
        CS = nb // 4
        for b in range(bloc):
            fat_sb[b] = fpool.tile([P, DT, na], F16, tag=f"fat{b}",
                                   name=f"fat_sb{b}")
            fbt_sb[b] = fpool.tile([P, DT, nb], F16, tag=f"fbt{b}",
                                   name=f"fbt_sb{b}")
        for b in range(bloc):
            for cslice in range(4):
                c0 = cslice * CS
                nc.sync.dma_start(
                    out=fbt_sb[b][:, :, c0:c0 + CS],
                    in_=fbtT[b, :, :, c0:c0 + CS].rearrange("j p c -> p j c"))
                nc.sync.dma_start(
                    out=fat_sb[b][:, :, c0:c0 + CS],
                    in_=fatT[b, :, :, c0:c0 + CS].rearrange("j p c -> p j c"))

        # ---- per-batch state tiles ----
        yb_dram, negd, t_all, dw, ya_all = {}, {}, {}, {}, {}
        for b in range(bloc):
            yb_dram[b] = dpool.tile([nb, D], F16, tag=f"ybd{b}",
                                    name=f"yb_dram{b}")
            negd[b] = tkpool.tile([P, nt, 8], F32, tag=f"negd{b}",
                                  name=f"negd{b}")
            t_all[b] = tkpool.tile([P, nt, 32], I16, tag=f"tall{b}",
                                   name=f"t_all{b}")
            dw[b] = tkpool.tile([P, nt, KNN], F32, tag=f"dw{b}",
                                name=f"dw{b}")
            ya_all[b] = yapool.tile([P, nt, D], F16, tag=f"ya{b}",
                                    name=f"ya_all{b}")

        # ---------- emission helpers ----------
        live_dist = {}
        live_h16 = {}

        def emit_dist(b, i, h):
            ps = dps.tile([P, HALF], F32, tag="dist", name="dist_ps")
            live_dist[(b, i, h)] = ps
            for q in range(2):
                c0 = h * HALF + q * 512
                nc.tensor.matmul(
                    out=ps[:, q * 512:(q + 1) * 512],
                    lhsT=phia_sb[b][:, i * P:(i + 1) * P],
                    rhs=phib_sb[b][:, c0:c0 + 512],
                    start=True, stop=True)

        def emit_scan(b, i, h):
            if h == 0:
                live_h16[(b, i)] = hpool.tile([P, 16], F32, tag="h16",
                                              name="h16")
            h16 = live_h16[(b, i)]
            nc.vector.max(out=h16[:, h * 8:(h + 1) * 8],
                          in_=live_dist.pop((b, i, h)))
            if h == 1:
                nc.vector.max(out=negd[b][:, i, :],
                              in_=live_h16.pop((b, i)))

        def emit_feat(b, kind, i):
            ps = mmps.tile([P, D], F32, tag="mm", name=f"{kind}_ps")
            src = fbt_sb[b] if kind == "yb" else fat_sb[b]
            wmat = wd_sb if kind == "yb" else w2_sb
            for j in range(DT):
                nc.tensor.matmul(
                    out=ps, lhsT=src[:, j, i * P:(i + 1) * P],
                    rhs=wmat[:, j, :],
                    start=(j == 0),
                    stop=(j == DT - 1 and not (kind == "ya" and with_bias)))
            if kind == "ya" and with_bias:
                nc.tensor.matmul(out=ps, lhsT=ones_t, rhs=bias_sb,
                                 start=False, stop=True)
            if kind == "yb":
                yb_sb = iopool.tile([P, D], F16, tag="ybsb", name="yb_sb")
                nc.scalar.copy(out=yb_sb, in_=ps)
                nc.sync.dma_start(out=yb_dram[b][i * P:(i + 1) * P, :],
                                  in_=yb_sb)
            else:
                nc.scalar.copy(out=ya_all[b][:, i, :], in_=ps)

        def emit_build(b, g0, glen):
            """Unpack idx/dw for tiles [g0, g0+glen) and build the wrapped
            int16 gather index list in t_all[b]."""
            nsl = slice(g0, g0 + glen)
            # packed = -(dist2 + m/2048); y = 2048*dist2 + m (exact int);
            # idx = y & 2047; dw = relu(R - sqrt((y - idx) * 2^-25))
            y_f = epool.tile([P, glen, KNN], F32, tag="y_f", name="y_f")
            nc.vector.tensor_scalar(
                out=y_f, in0=negd[b][:, nsl, 0:KNN], scalar1=-2048.0,
                scalar2=None, op0=ALU.mult)
            y_u = epool.tile([P, glen, KNN], U32, tag="y_u", name="y_u")
            nc.vector.tensor_copy(out=y_u, in_=y_f)
            idx_u = epool.tile([P, glen, KNN], U32, tag="idx_u", name="idx_u")
            nc.vector.tensor_scalar(
                out=idx_u, in0=y_u, scalar1=2047, scalar2=None,
                op0=ALU.bitwise_and)
            d2_f = epool.tile([P, glen, KNN], F32, tag="d2_f", name="d2_f")
            nc.vector.tensor_tensor(out=d2_f, in0=y_u, in1=idx_u,
                                    op=ALU.subtract)
            dist4 = epool.tile([P, glen, KNN], F32, tag="dist4", name="dist4")
            nc.scalar.activation(out=dist4, in_=d2_f, func=AF.Sqrt,
                                 scale=float(2.0 ** -25))
            nc.scalar.activation(out=dw[b][:, nsl, :], in_=dist4, func=AF.Relu,
                                 scale=-1.0, bias=rconst)

            # wrapped idx list: t_all[16g+c, i, 8k+pp] = idx[16pp+c, i, k]
            # for every group g (PE transposes + one-hot replicate matmul).
            nq = glen * KNN
            idxf = epool.tile([P, nq], F16, tag="idxf", name="idxf")
            nc.vector.tensor_copy(
                out=idxf, in_=idx_u[:].rearrange("p i k -> p (i k)"))
            tpx0 = tpps.tile([nt * KNN, P], F16, tag="tpx", name="idxT_ps")
            idxT_ps = tpx0[0:nq, :]
            nc.tensor.transpose(out=idxT_ps, in_=idxf, identity=ident_sb)
            idxT_sb = epool.tile([nt * KNN, P], F16, tag="idxT_sb",
                                 name="idxT_sb")
            nc.scalar.copy(out=idxT_sb[0:nq, :], in_=idxT_ps)
            t16 = epool.tile([16, nt, 32], F16, tag="t16", name="t16")
            for pp in range(8):
                tp_big = tpps.tile([nt * KNN, P], F16, tag="tpx",
                                   name="tp_ps")
                tp_ps = tp_big[0:16, 0:nq]
                nc.tensor.transpose(
                    out=tp_ps, in_=idxT_sb[0:nq, 16 * pp:16 * (pp + 1)],
                    identity=ident_sb[0:nq, 0:nq])
                nc.scalar.copy(
                    out=t16[:, 0:glen, pp::8],
                    in_=tp_ps.rearrange("c (i k) -> c i k", i=glen, k=KNN))
            rep_full = mmps.tile([P, D], F32, tag="mm", name="rep_ps")
            rep_ps = rep_full[:, 0:glen * 32]
            nc.tensor.matmul(
                out=rep_ps, lhsT=rep_sb,
                rhs=t16[:, 0:glen, :].rearrange("c i j -> c (i j)"),
                start=True, stop=True)
            nc.scalar.copy(
                out=t_all[b][:, nsl, :].rearrange("p i j -> p (i j)"),
                in_=rep_ps)

        live_gather = {}
        gather_count = [0]

        def emit_gather(b, ig):
            ybg = gpool.tile([P, GW, KNN, D], F16, tag="ybg", name="ybg")
            live_gather[(b, ig)] = ybg
            qn = gather_count[0] % N_QUEUES
            gather_count[0] += 1
            nc.gpsimd.dma_gather(
                out_ap=ybg[:].rearrange("p g k d -> p (g k) d"),
                in_ap=yb_dram[b][:],
                idxs_ap=t_all[b][:, ig:ig + GW, :].rearrange(
                    "p g j -> p (g j)"),
                num_idxs=P * KNN * GW, num_idxs_reg=P * KNN * GW,
                elem_size=D, queue_num=qn)

        live_r = {}

        def emit_combine_z(b, i):
            ig = (i // GW) * GW
            ybgi = live_gather[(b, ig)][:, i - ig]
            if i - ig == GW - 1:
                del live_gather[(b, ig)]
            z = mpool.tile([P, KNN, D], F16, tag="z", name="z")
            ya_b = ya_all[b][:, i, :].unsqueeze(1).broadcast_to([P, KNN, D])
            nc.vector.tensor_tensor(out=z, in0=ybgi, in1=ya_b, op=ALU.add)
            r = mpool.tile([P, KNN, D], F16, tag="r", name="r")
            live_r[(b, i)] = r
            for k in range(KNN):
                nc.scalar.activation(
                    out=r[:, k, :], in_=z[:, k, :],
                    func=AF.Relu, scale=dw[b][:, i, k:k + 1])

        live_fused = {}

        def emit_combine_s(b, i):
            r = live_r.pop((b, i))
            s01 = mpool.tile([P, 2, D], F16, tag="s01", name="s01")
            nc.vector.tensor_tensor(out=s01, in0=r[:, 0:2, :],
                                    in1=r[:, 2:4, :], op=ALU.add)
            fused = fpool2.tile([P, D], F16, tag="fused", name="fused")
            live_fused[(b, i)] = fused
            nc.vector.tensor_add(fused, s01[:, 0, :], s01[:, 1, :])

        def emit_out(b, i):
            # out-DMAs issue on the (FIFO) sync queue strictly after every
            # yb_dram write: a not-yet-ready fused tile at the queue head
            # would otherwise stall the yb writes behind it, which delays the
            # first gather and cascades through the whole pipeline.
            nc.sync.dma_start(out=out[b, i * P:(i + 1) * P, :],
                              in_=live_fused.pop((b, i)))

        # ---------- static schedule ----------
        # dist halves in batch-major order; feature tiles: yb b0, ya b0,
        # yb b1, ya b1.
        dist_halves = [(b, i, h) for b in range(bloc) for i in range(nt)
                       for h in range(2)]
        feats = []
        for b in range(bloc):
            feats += [(b, "yb", i) for i in range(nt)]
            feats += [(b, "ya", i) for i in range(nt)]
        # builds due after the last scan round of their tile group
        build_at = {}
        for b in range(bloc):
            for (g0, glen) in GROUPS[b]:
                last_round = b * 2 * nt + (g0 + glen) * 2  # rounds consumed
                build_at.setdefault(last_round, []).append((b, g0, glen))

        # gather calls in issue order; each covers GW tiles. Combines must
        # trail their gather closely in emission order (the 3-deep ybg ring
        # means gather g+3 reuses g's slot; a reader emitted later than that
        # writer is an unenforced WAR hazard -> data corruption). Each
        # combine is split into two ~1.2us DVE insertions so the scan stream
        # never stalls long enough to backpressure the PE off its p-state.
        gather_sched = []   # (round_due, b, ig)
        combine_sched = []  # (round_due, kind, b, i)
        for b in range(bloc):
            for (g0, glen) in GROUPS[b]:
                due = b * 2 * nt + (g0 + glen) * 2 + 1
                for j, ig in enumerate(range(g0, g0 + glen, GW)):
                    gather_sched.append((max(due, 22) + 3 * j, b, ig))
                    cd = max(due, 22) + 3 * j
                    combine_sched.append((cd + 4, 'z', b, ig))
                    combine_sched.append((cd + 5, 's', b, ig))
                    combine_sched.append((cd + 6, 'z', b, ig + 1))
                    combine_sched.append((cd + 7, 's', b, ig + 1))
        # out-DMAs: after the last yb_dram write round, in combine order
        last_yb_round = (bloc - 1) * 2 * nt + nt + 1
        out_sched = [(max(c[0] + 2, last_yb_round + jj), c[2], c[3])
                     for jj, c in enumerate(
                         [c for c in combine_sched if c[1] == 's'])]

        n_rounds = len(dist_halves)
        gi, ci, oi = 0, 0, 0
        for r in range(n_rounds + 60):
            if r < n_rounds:
                b, i, h = dist_halves[r]
                emit_dist(b, i, h)
                emit_feat(*feats[r])
                emit_scan(b, i, h)
            for (b, g0, glen) in build_at.get(r + 1, []):
                emit_build(b, g0, glen)
            while gi < len(gather_sched) and gather_sched[gi][0] <= r + 1:
                emit_gather(gather_sched[gi][1], gather_sched[gi][2])
                gi += 1
            while ci < len(combine_sched) and combine_sched[ci][0] <= r + 1:
                _, kind, cb, cti = combine_sched[ci]
                if kind == 'z':
                    emit_combine_z(cb, cti)
                else:
                    emit_combine_s(cb, cti)
                ci += 1
            while oi < len(out_sched) and out_sched[oi][0] <= r + 1:
                emit_out(out_sched[oi][1], out_sched[oi][2])
                oi += 1
        assert gi == len(gather_sched) and ci == len(combine_sched) \
            and oi == len(out_sched), (gi, ci, oi)


# ---------------------------------------------------------------------------
# host side
# ---------------------------------------------------------------------------

def _host_inputs(feats_a, feats_b, W, bias, coords_a, coords_b):
    """Host-side prep: fp16 casts, chunk transposes, lifted packed coords."""
    d = W.shape[1]
    dt = d // P
    bsz, na_, _ = feats_a.shape
    nb_ = feats_b.shape[1]

    ca = coords_a.astype(np.int64)
    cb = coords_b.astype(np.int64)
    a2 = (ca * ca).sum(-1)                      # [B, Na] ints < 48388
    b2 = (cb * cb).sum(-1)
    hiA, loA = a2 >> 11, a2 & 2047
    hiB, loB = b2 >> 11, b2 & 2047
    m_over = (np.arange(nb_, dtype=np.float32) / 2048.0)[None, :].repeat(bsz, 0)
    # packed dot = 2a.b - |a|^2 - |b|^2 - m/2048 = -(dist2 + m/2048)
    phia8 = np.stack([ca[..., 0], ca[..., 1], ca[..., 2], hiA, loA,
                      2048 * np.ones((bsz, na_), np.int64),
                      np.ones((bsz, na_), np.int64),
                      np.ones((bsz, na_), np.int64)], axis=1).astype(np.float16)
    phib8 = np.stack([2.0 * cb[..., 0], 2.0 * cb[..., 1], 2.0 * cb[..., 2],
                      -2048 * np.ones((bsz, nb_), np.float64),
                      -np.ones((bsz, nb_), np.float64),
                      -hiB.astype(np.float64), -loB.astype(np.float64),
                      -m_over.astype(np.float64)], axis=1).astype(np.float16)

    # feats chunk-transposed: [B, dt, 128, N]
    fatT = np.ascontiguousarray(
        feats_a.reshape(bsz, na_, dt, P).transpose(0, 2, 3, 1)).astype(np.float16)
    fbtT = np.ascontiguousarray(
        feats_b.reshape(bsz, nb_, dt, P).transpose(0, 2, 3, 1)).astype(np.float16)

    w2f = W[d:]                                  # applies to a_f
    wdf = W[:d] - W[d:]                          # applies to b_f
    w2c = np.ascontiguousarray(w2f.reshape(dt, P, d)).astype(np.float16)
    wdc = np.ascontiguousarray(wdf.reshape(dt, P, d)).astype(np.float16)
    biasw = bias.reshape(1, d).astype(np.float16)
    return fatT, fbtT, phia8, phib8, w2c, wdc, biasw


def kernel(**inputs):
    feats_a = np.asarray(inputs["feats_a"], dtype=np.float32)
    feats_b = np.asarray(inputs["feats_b"], dtype=np.float32)
    W = np.asarray(inputs["W"], dtype=np.float32)
    bias = np.asarray(inputs["bias"], dtype=np.float32)
    coords_a = np.asarray(inputs["coords_a"])
    coords_b = np.asarray(inputs["coords_b"])

    fatT, fbtT, phia8, phib8, w2c, wdc, biasw = _host_inputs(
        feats_a, feats_b, W, bias, coords_a, coords_b)
    with_bias = bool(np.any(bias != 0.0))
    identm = np.eye(P, dtype=np.float16)
    repm = np.zeros((16, P), np.float16)
    for p_ in range(P):
        repm[p_ % 16, p_] = 1.0

    nc = build_bass(with_bias=with_bias)

    in_maps = []
    for c in range(N_CORES):
        s = slice(c * BLOC, (c + 1) * BLOC)
        in_maps.append({
            "fatT": np.ascontiguousarray(fatT[s]),
            "fbtT": np.ascontiguousarray(fbtT[s]),
            "phia": np.ascontiguousarray(phia8[s]),
            "phib": np.ascontiguousarray(phib8[s]),
            "w2": w2c,
            "wd": wdc,
            "biasw": biasw,
            "ident": identm,
            "rep": repm,
        })

    from concourse import bass_utils
    res = bass_utils.run_bass_kernel_spmd(nc, in_maps, core_ids=list(range(N_CORES)))
    fused = np.concatenate([r["out"] for r in res.results], axis=0)
    return np.concatenate([feats_a, fused.astype(np.float32)], axis=-1)


if __name__ == "__main__":
    nc = build_bass()
    print("built ok")


# revision 24
# speedup vs baseline: 1.1456x; 1.1456x over previous
"""Trainium2 Bass kernel for retrieval_knn (nn_CLI_v1_63702954934484).

Reference computation (per batch b):
    dist[n,m] = ||ca[n] - cb[m]|| / 128                         [Na, Nb]
    idx       = argtop4-smallest(dist[n,:])                     [Na, 4]
    dw        = R - clip(dist_top4, 0, R)                       [Na, 4]
    h         = [b_f, a_f - b_f]  (b_f = feats_b[idx])          [Na, 4, 2D]
    fused     = sum_k relu(h @ W + bias) * dw                   [Na, D]
    out       = [feats_a, fused]                                [Na, 2D]

v3: single flat software pipeline per core (2 batches), emission-ordered
so every engine runs back-to-back:
  * PE: 64 "rounds", each = 1 distance half-tile (2x512-col fp16 matmuls,
    packed -(dist2+m/2048) values) + 1 feature tile (Yb or Ya, 4 matmuls).
    Continuous occupancy keeps the PE at the 2.4GHz p-state (it drops to
    1.2GHz whenever the engine idles and needs ~3us busy to re-ramit).
  * DVE: max8 scans trail the dist matmuls; z-adds and k-sums for the MLP
    combine are interleaved between later scans.
  * Act: PSUM->SBUF fp16 copies of Ya/Yb + relu(z)*dw (activation scale).
  * Pool: Q7 dma_gather of Yb rows (2 tiles / 1024 idx per call), round-
    robined over 4 SWDGE queues so descriptor rings never force a drain.
  * top-4 via max8 on packed fp32 PSUM: value = -(dist2 + m/2048) gives
    bit-exact jax.lax.top_k ordering incl. index tie-breaks; indices are
    unpacked arithmetically (no max_index rescan).

Sharding: data-parallel over batch (16 batches -> 8 cores x 2).
"""

import sys

sys.path.insert(0, "/opt/trn_rl_repo")

import numpy as np

import concourse.bass as bass
import concourse.mybir as mybir
import concourse.tile as tile
from concourse import bacc
from concourse.library_config import mlp as mlp_lib

F32 = mybir.dt.float32
F16 = mybir.dt.float16
U32 = mybir.dt.uint32
I16 = mybir.dt.int16

P = 128          # partitions
D = 512          # feature dim
KNN = 4          # neighbors
R = 0.5
FULL_SCALE = 128.0

B = 16           # full batch
N_CORES = 8
BLOC = B // N_CORES  # batches per core

NA = 2048
NB = 2048
NT = NA // P     # n-tiles per batch
DT = D // P      # 128-chunks of the feature dim
HALF = 1024      # distance column chunk (2 PSUM banks)

AF = mybir.ActivationFunctionType
ALU = mybir.AluOpType

N_QUEUES = 4     # SWDGE queues for dma_gather round-robin
GW = 2           # tiles per dma_gather call

# idx-build groups per batch: (start_tile, num_tiles). First b0 group is
# small so the first gather can be issued early (Pool is the critical
# engine; it must start as soon as possible).
GROUPS = {0: [(0, 8), (8, 8)], 1: [(0, 8), (8, 8)]}


def build_bass(bloc=BLOC, na=NA, nb=NB, with_bias=False):
    nc = bacc.Bacc("TRN2", debug=False, num_swdge_queues=N_QUEUES)
    fatT = nc.dram_tensor("fatT", [bloc, DT, P, na], F16, kind="ExternalInput").ap()
    fbtT = nc.dram_tensor("fbtT", [bloc, DT, P, nb], F16, kind="ExternalInput").ap()
    phia = nc.dram_tensor("phia", [bloc, 8, na], F16, kind="ExternalInput").ap()
    phib = nc.dram_tensor("phib", [bloc, 8, nb], F16, kind="ExternalInput").ap()
    w2 = nc.dram_tensor("w2", [DT, P, D], F16, kind="ExternalInput").ap()
    wd = nc.dram_tensor("wd", [DT, P, D], F16, kind="ExternalInput").ap()
    biasw = nc.dram_tensor("biasw", [1, D], F16, kind="ExternalInput").ap()
    ident = nc.dram_tensor("ident", [P, P], F16, kind="ExternalInput").ap()
    rep = nc.dram_tensor("rep", [16, P], F16, kind="ExternalInput").ap()
    out = nc.dram_tensor("out", [bloc, na, D], F16, kind="ExternalOutput").ap()

    nc.gpsimd.load_library(mlp_lib)
    with tile.TileContext(nc) as tc:
        _kern(tc, fatT, fbtT, phia, phib, w2, wd, biasw, ident, rep, out,
              bloc=bloc, na=na, nb=nb, with_bias=with_bias)
    nc.compile()
    return nc


def _kern(tc, fatT, fbtT, phia, phib, w2, wd, biasw, ident, rep, out, *,
          bloc, na, nb, with_bias):
    nc = tc.nc
    nt = na // P

    with (
        tc.tile_pool(name="const", bufs=1) as cpool,
        tc.tile_pool(name="wpool", bufs=1) as wpool,
        tc.tile_pool(name="feat", bufs=1) as fpool,
        tc.tile_pool(name="phi", bufs=1) as phipool,
        tc.tile_pool(name="tk", bufs=1) as tkpool,
        tc.tile_pool(name="ext", bufs=2) as epool,
        tc.tile_pool(name="h16p", bufs=3) as hpool,
        tc.tile_pool(name="io", bufs=3) as iopool,
        tc.tile_pool(name="ya", bufs=1) as yapool,
        tc.tile_pool(name="gat", bufs=3) as gpool,
        tc.tile_pool(name="mlp", bufs=2) as mpool,
        tc.tile_pool(name="fusedp", bufs=10) as fpool2,
        tc.tile_pool(name="dram", bufs=1, space="DRAM") as dpool,
        tc.tile_pool(name="dist_ps", bufs=2, space="PSUM") as dps,
        tc.tile_pool(name="mm_ps", bufs=3, space="PSUM") as mmps,
        tc.tile_pool(name="tp_ps", bufs=1, space="PSUM") as tpps,
    ):
        rconst = cpool.tile([P, 1], F32, name="rconst")
        nc.vector.memset(rconst, R)
        ones_t = cpool.tile([1, P], F16, name="ones_t")
        nc.vector.memset(ones_t, 1.0)

        # resident weights (fp16)
        w2_sb = wpool.tile([P, DT, D], F16, name="w2_sb")
        wd_sb = wpool.tile([P, DT, D], F16, name="wd_sb")
        for j in range(DT):
            nc.sync.dma_start(out=w2_sb[:, j, :], in_=w2[j])
            nc.sync.dma_start(out=wd_sb[:, j, :], in_=wd[j])
        bias_sb = wpool.tile([1, D], F16, name="bias_sb")
        if with_bias:
            nc.sync.dma_start(out=bias_sb, in_=biasw)
        ident_sb = wpool.tile([P, P], F16, name="ident_sb")
        rep_sb = wpool.tile([16, P], F16, name="rep_sb")
        nc.sync.dma_start(out=ident_sb, in_=ident)
        nc.sync.dma_start(out=rep_sb, in_=rep)

        # ---- upfront loads for both batches ----
        phia_sb, phib_sb, fat_sb, fbt_sb = {}, {}, {}, {}
        for b in range(bloc):
            phia_sb[b] = phipool.tile([8, na], F16, tag=f"phia{b}",
                                      name=f"phia_sb{b}")
            phib_sb[b] = phipool.tile([8, nb], F16, tag=f"phib{b}",
                                      name=f"phib_sb{b}")
            nc.sync.dma_start(out=phia_sb[b], in_=phia[b])
            nc.sync.dma_start(out=phib_sb[b], in_=phib[b])
        CS = nb // 4
        for b in range(bloc):
            fat_sb[b] = fpool.tile([P, DT, na], F16, tag=f"fat{b}",
                                   name=f"fat_sb{b}")
            fbt_sb[b] = fpool.tile([P, DT, nb], F16, tag=f"fbt{b}",
                                   name=f"fbt_sb{b}")
        for b in range(bloc):
            for cslice in range(4):
                c0 = cslice * CS
                nc.sync.dma_start(
                    out=fbt_sb[b][:, :, c0:c0 + CS],
                    in_=fbtT[b, :, :, c0:c0 + CS].rearrange("j p c -> p j c"))
                nc.sync.dma_start(
                    out=fat_sb[b][:, :, c0:c0 + CS],
                    in_=fatT[b, :, :, c0:c0 + CS].rearrange("j p c -> p j c"))

        # ---- per-batch state tiles ----
        yb_dram, negd, t_all, dw, ya_all = {}, {}, {}, {}, {}
        for b in range(bloc):
            yb_dram[b] = dpool.tile([nb, D], F16, tag=f"ybd{b}",
                                    name=f"yb_dram{b}")
            negd[b] = tkpool.tile([P, nt, 8], F32, tag=f"negd{b}",
                                  name=f"negd{b}")
            t_all[b] = tkpool.tile([P, nt, 32], I16, tag=f"tall{b}",
                                   name=f"t_all{b}")
            dw[b] = tkpool.tile([P, nt, KNN], F32, tag=f"dw{b}",
                                name=f"dw{b}")
            ya_all[b] = yapool.tile([P, nt, D], F16, tag=f"ya{b}",
                                    name=f"ya_all{b}")

        # ---------- emission helpers ----------
        live_dist = {}
        live_h16 = {}

        def emit_dist(b, i, h):
            ps = dps.tile([P, HALF], F32, tag="dist", name="dist_ps")
            live_dist[(b, i, h)] = ps
            for q in range(2):
                c0 = h * HALF + q * 512
                nc.tensor.matmul(
                    out=ps[:, q * 512:(q + 1) * 512],
                    lhsT=phia_sb[b][:, i * P:(i + 1) * P],
                    rhs=phib_sb[b][:, c0:c0 + 512],
                    start=True, stop=True)

        def emit_scan(b, i, h):
            if h == 0:
                live_h16[(b, i)] = hpool.tile([P, 16], F32, tag="h16",
                                              name="h16")
            h16 = live_h16[(b, i)]
            nc.vector.max(out=h16[:, h * 8:(h + 1) * 8],
                          in_=live_dist.pop((b, i, h)))
            if h == 1:
                nc.vector.max(out=negd[b][:, i, :],
                              in_=live_h16.pop((b, i)))

        def emit_feat(b, kind, i):
            ps = mmps.tile([P, D], F32, tag="mm", name=f"{kind}_ps")
            src = fbt_sb[b] if kind == "yb" else fat_sb[b]
            wmat = wd_sb if kind == "yb" else w2_sb
            for j in range(DT):
                nc.tensor.matmul(
                    out=ps, lhsT=src[:, j, i * P:(i + 1) * P],
                    rhs=wmat[:, j, :],
                    start=(j == 0),
                    stop=(j == DT - 1 and not (kind == "ya" and with_bias)))
            if kind == "ya" and with_bias:
                nc.tensor.matmul(out=ps, lhsT=ones_t, rhs=bias_sb,
                                 start=False, stop=True)
            if kind == "yb":
                yb_sb = iopool.tile([P, D], F16, tag="ybsb", name="yb_sb")
                nc.scalar.copy(out=yb_sb, in_=ps)
                nc.sync.dma_start(out=yb_dram[b][i * P:(i + 1) * P, :],
                                  in_=yb_sb)
            else:
                nc.scalar.copy(out=ya_all[b][:, i, :], in_=ps)

        def emit_build(b, g0, glen):
            """Unpack idx/dw for tiles [g0, g0+glen) and build the wrapped
            int16 gather index list in t_all[b]."""
            nsl = slice(g0, g0 + glen)
            # packed = -(dist2 + m/2048); y = 2048*dist2 + m (exact int);
            # idx = y & 2047; dw = relu(R - sqrt((y - idx) * 2^-25))
            y_f = epool.tile([P, glen, KNN], F32, tag="y_f", name="y_f")
            nc.vector.tensor_scalar(
                out=y_f, in0=negd[b][:, nsl, 0:KNN], scalar1=-2048.0,
                scalar2=None, op0=ALU.mult)
            y_u = epool.tile([P, glen, KNN], U32, tag="y_u", name="y_u")
            nc.vector.tensor_copy(out=y_u, in_=y_f)
            idx_u = epool.tile([P, glen, KNN], U32, tag="idx_u", name="idx_u")
            nc.vector.tensor_scalar(
                out=idx_u, in0=y_u, scalar1=2047, scalar2=None,
                op0=ALU.bitwise_and)
            d2_f = epool.tile([P, glen, KNN], F32, tag="d2_f", name="d2_f")
            nc.vector.tensor_tensor(out=d2_f, in0=y_u, in1=idx_u,
                                    op=ALU.subtract)
            dist4 = epool.tile([P, glen, KNN], F32, tag="dist4", name="dist4")
            nc.scalar.activation(out=dist4, in_=d2_f, func=AF.Sqrt,
                                 scale=float(2.0 ** -25))
            nc.scalar.activation(out=dw[b][:, nsl, :], in_=dist4, func=AF.Relu,
                                 scale=-1.0, bias=rconst)

            # wrapped idx list: t_all[16g+c, i, 8k+pp] = idx[16pp+c, i, k]
            # for every group g (PE transposes + one-hot replicate matmul).
            nq = glen * KNN
            idxf = epool.tile([P, nq], F16, tag="idxf", name="idxf")
            nc.vector.tensor_copy(
                out=idxf, in_=idx_u[:].rearrange("p i k -> p (i k)"))
            tpx0 = tpps.tile([nt * KNN, P], F16, tag="tpx", name="idxT_ps")
            idxT_ps = tpx0[0:nq, :]
            nc.tensor.transpose(out=idxT_ps, in_=idxf, identity=ident_sb)
            idxT_sb = epool.tile([nt * KNN, P], F16, tag="idxT_sb",
                                 name="idxT_sb")
            nc.scalar.copy(out=idxT_sb[0:nq, :], in_=idxT_ps)
            t16 = epool.tile([16, nt, 32], F16, tag="t16", name="t16")
            for pp in range(8):
                tp_big = tpps.tile([nt * KNN, P], F16, tag="tpx",
                                   name="tp_ps")
                tp_ps = tp_big[0:16, 0:nq]
                nc.tensor.transpose(
                    out=tp_ps, in_=idxT_sb[0:nq, 16 * pp:16 * (pp + 1)],
                    identity=ident_sb[0:nq, 0:nq])
                nc.scalar.copy(
                    out=t16[:, 0:glen, pp::8],
                    in_=tp_ps.rearrange("c (i k) -> c i k", i=glen, k=KNN))
            rep_full = mmps.tile([P, D], F32, tag="mm", name="rep_ps")
            rep_ps = rep_full[:, 0:glen * 32]
            nc.tensor.matmul(
                out=rep_ps, lhsT=rep_sb,
                rhs=t16[:, 0:glen, :].rearrange("c i j -> c (i j)"),
                start=True, stop=True)
            nc.scalar.copy(
                out=t_all[b][:, nsl, :].rearrange("p i j -> p (i j)"),
                in_=rep_ps)

        live_gather = {}
        gather_count = [0]

        def emit_gather(b, ig):
            ybg = gpool.tile([P, GW, KNN, D], F16, tag="ybg", name="ybg")
            live_gather[(b, ig)] = ybg
            qn = gather_count[0] % N_QUEUES
            gather_count[0] += 1
            nc.gpsimd.dma_gather(
                out_ap=ybg[:].rearrange("p g k d -> p (g k) d"),
                in_ap=yb_dram[b][:],
                idxs_ap=t_all[b][:, ig:ig + GW, :].rearrange(
                    "p g j -> p (g j)"),
                num_idxs=P * KNN * GW, num_idxs_reg=P * KNN * GW,
                elem_size=D, queue_num=qn)

        live_r = {}

        def emit_combine_z(b, i):
            ig = (i // GW) * GW
            ybgi = live_gather[(b, ig)][:, i - ig]
            if i - ig == GW - 1:
                del live_gather[(b, ig)]
            z = mpool.tile([P, KNN, D], F16, tag="z", name="z")
            ya_b = ya_all[b][:, i, :].unsqueeze(1).broadcast_to([P, KNN, D])
            nc.vector.tensor_tensor(out=z, in0=ybgi, in1=ya_b, op=ALU.add)
            r = mpool.tile([P, KNN, D], F16, tag="r", name="r")
            live_r[(b, i)] = r
            for k in range(KNN):
                nc.scalar.activation(
                    out=r[:, k, :], in_=z[:, k, :],
                    func=AF.Relu, scale=dw[b][:, i, k:k + 1])

        live_fused = {}

        def emit_combine_s(b, i):
            r = live_r.pop((b, i))
            s01 = mpool.tile([P, 2, D], F16, tag="s01", name="s01")
            nc.vector.tensor_tensor(out=s01, in0=r[:, 0:2, :],
                                    in1=r[:, 2:4, :], op=ALU.add)
            fused = fpool2.tile([P, D], F16, tag="fused", name="fused")
            live_fused[(b, i)] = fused
            nc.vector.tensor_add(fused, s01[:, 0, :], s01[:, 1, :])

        def emit_out(b, i):
            # out-DMAs issue on the (FIFO) sync queue strictly after every
            # yb_dram write: a not-yet-ready fused tile at the queue head
            # would otherwise stall the yb writes behind it, which delays the
            # first gather and cascades through the whole pipeline.
            nc.sync.dma_start(out=out[b, i * P:(i + 1) * P, :],
                              in_=live_fused.pop((b, i)))

        # ---------- static schedule ----------
        # dist halves in batch-major order; feature tiles: yb b0, ya b0,
        # yb b1, ya b1.
        dist_halves = [(b, i, h) for b in range(bloc) for i in range(nt)
                       for h in range(2)]
        feats = []
        for b in range(bloc):
            feats += [(b, "yb", i) for i in range(nt)]
            feats += [(b, "ya", i) for i in range(nt)]
        # builds due after the last scan round of their tile group
        build_at = {}
        for b in range(bloc):
            for (g0, glen) in GROUPS[b]:
                last_round = b * 2 * nt + (g0 + glen) * 2  # rounds consumed
                build_at.setdefault(last_round, []).append((b, g0, glen))

        # gather calls in issue order; each covers GW tiles. Combines must
        # trail their gather closely in emission order (the 3-deep ybg ring
        # means gather g+3 reuses g's slot; a reader emitted later than that
        # writer is an unenforced WAR hazard -> data corruption). Each
        # combine is split into two ~1.2us DVE insertions so the scan stream
        # never stalls long enough to backpressure the PE off its p-state.
        gather_sched = []   # (round_due, b, ig)
        combine_sched = []  # (round_due, kind, b, i)
        for b in range(bloc):
            for (g0, glen) in GROUPS[b]:
                due = b * 2 * nt + (g0 + glen) * 2 + 1
                for j, ig in enumerate(range(g0, g0 + glen, GW)):
                    gather_sched.append((max(due, 22) + 3 * j, b, ig))
                    cd = max(due, 22) + 3 * j
                    combine_sched.append((cd + 4, 'z', b, ig))
                    combine_sched.append((cd + 5, 's', b, ig))
                    combine_sched.append((cd + 6, 'z', b, ig + 1))
                    combine_sched.append((cd + 7, 's', b, ig + 1))
        # out-DMAs: after the last yb_dram write round, in combine order
        last_yb_round = (bloc - 1) * 2 * nt + nt + 1
        out_sched = [(max(c[0] + 2, last_yb_round + jj), c[2], c[3])
                     for jj, c in enumerate(
                         [c for c in combine_sched if c[1] == 's'])]

        n_rounds = len(dist_halves)
        gi, ci, oi = 0, 0, 0
        for r in range(n_rounds + 60):
            if r < n_rounds:
                b, i, h = dist_halves[r]
                emit_dist(b, i, h)
                # feature tiles front-loaded 2/round: the Act queue is pure
                # PSUM->SBUF copies early (keeps the PE's mm ring drained and
                # the PE continuously busy = fast p-state); the back half of
                # the rounds is then dist/scan/combine with a near-idle PE.
                if 2 * r < len(feats):
                    emit_feat(*feats[2 * r])
                if 2 * r + 1 < len(feats):
                    emit_feat(*feats[2 * r + 1])
                emit_scan(b, i, h)
            for (b, g0, glen) in build_at.get(r + 1, []):
                emit_build(b, g0, glen)
            while gi < len(gather_sched) and gather_sched[gi][0] <= r + 1:
                emit_gather(gather_sched[gi][1], gather_sched[gi][2])
                gi += 1
            while ci < len(combine_sched) and combine_sched[ci][0] <= r + 1:
                _, kind, cb, cti = combine_sched[ci]
                if kind == 'z':
                    emit_combine_z(cb, cti)
                else:
                    emit_combine_s(cb, cti)
                ci += 1
            while oi < len(out_sched) and out_sched[oi][0] <= r + 1:
                emit_out(out_sched[oi][1], out_sched[oi][2])
                oi += 1
        assert gi == len(gather_sched) and ci == len(combine_sched) \
            and oi == len(out_sched), (gi, ci, oi)


# ---------------------------------------------------------------------------
# host side
# ---------------------------------------------------------------------------

def _host_inputs(feats_a, feats_b, W, bias, coords_a, coords_b):
    """Host-side prep: fp16 casts, chunk transposes, lifted packed coords."""
    d = W.shape[1]
    dt = d // P
    bsz, na_, _ = feats_a.shape
    nb_ = feats_b.shape[1]

    ca = coords_a.astype(np.int64)
    cb = coords_b.astype(np.int64)
    a2 = (ca * ca).sum(-1)                      # [B, Na] ints < 48388
    b2 = (cb * cb).sum(-1)
    hiA, loA = a2 >> 11, a2 & 2047
    hiB, loB = b2 >> 11, b2 & 2047
    m_over = (np.arange(nb_, dtype=np.float32) / 2048.0)[None, :].repeat(bsz, 0)
    # packed dot = 2a.b - |a|^2 - |b|^2 - m/2048 = -(dist2 + m/2048)
    phia8 = np.stack([ca[..., 0], ca[..., 1], ca[..., 2], hiA, loA,
                      2048 * np.ones((bsz, na_), np.int64),
                      np.ones((bsz, na_), np.int64),
                      np.ones((bsz, na_), np.int64)], axis=1).astype(np.float16)
    phib8 = np.stack([2.0 * cb[..., 0], 2.0 * cb[..., 1], 2.0 * cb[..., 2],
                      -2048 * np.ones((bsz, nb_), np.float64),
                      -np.ones((bsz, nb_), np.float64),
                      -hiB.astype(np.float64), -loB.astype(np.float64),
                      -m_over.astype(np.float64)], axis=1).astype(np.float16)

    # feats chunk-transposed: [B, dt, 128, N]
    fatT = np.ascontiguousarray(
        feats_a.reshape(bsz, na_, dt, P).transpose(0, 2, 3, 1)).astype(np.float16)
    fbtT = np.ascontiguousarray(
        feats_b.reshape(bsz, nb_, dt, P).transpose(0, 2, 3, 1)).astype(np.float16)

    w2f = W[d:]                                  # applies to a_f
    wdf = W[:d] - W[d:]                          # applies to b_f
    w2c = np.ascontiguousarray(w2f.reshape(dt, P, d)).astype(np.float16)
    wdc = np.ascontiguousarray(wdf.reshape(dt, P, d)).astype(np.float16)
    biasw = bias.reshape(1, d).astype(np.float16)
    return fatT, fbtT, phia8, phib8, w2c, wdc, biasw


def kernel(**inputs):
    feats_a = np.asarray(inputs["feats_a"], dtype=np.float32)
    feats_b = np.asarray(inputs["feats_b"], dtype=np.float32)
    W = np.asarray(inputs["W"], dtype=np.float32)
    bias = np.asarray(inputs["bias"], dtype=np.float32)
    coords_a = np.asarray(inputs["coords_a"])
    coords_b = np.asarray(inputs["coords_b"])

    fatT, fbtT, phia8, phib8, w2c, wdc, biasw = _host_inputs(
        feats_a, feats_b, W, bias, coords_a, coords_b)
    with_bias = bool(np.any(bias != 0.0))
    identm = np.eye(P, dtype=np.float16)
    repm = np.zeros((16, P), np.float16)
    for p_ in range(P):
        repm[p_ % 16, p_] = 1.0

    nc = build_bass(with_bias=with_bias)

    in_maps = []
    for c in range(N_CORES):
        s = slice(c * BLOC, (c + 1) * BLOC)
        in_maps.append({
            "fatT": np.ascontiguousarray(fatT[s]),
            "fbtT": np.ascontiguousarray(fbtT[s]),
            "phia": np.ascontiguousarray(phia8[s]),
            "phib": np.ascontiguousarray(phib8[s]),
            "w2": w2c,
            "wd": wdc,
            "biasw": biasw,
            "ident": identm,
            "rep": repm,
        })

    from concourse import bass_utils
    res = bass_utils.run_bass_kernel_spmd(nc, in_maps, core_ids=list(range(N_CORES)))
    fused = np.concatenate([r["out"] for r in res.results], axis=0)
    return np.concatenate([feats_a, fused.astype(np.float32)], axis=-1)


if __name__ == "__main__":
    nc = build_bass()
    print("built ok")


# revision 25
# speedup vs baseline: 1.2786x; 1.1161x over previous
"""Trainium2 Bass kernel for retrieval_knn (nn_CLI_v1_63702954934484).

Reference computation (per batch b):
    dist[n,m] = ||ca[n] - cb[m]|| / 128                         [Na, Nb]
    idx       = argtop4-smallest(dist[n,:])                     [Na, 4]
    dw        = R - clip(dist_top4, 0, R)                       [Na, 4]
    h         = [b_f, a_f - b_f]  (b_f = feats_b[idx])          [Na, 4, 2D]
    fused     = sum_k relu(h @ W + bias) * dw                   [Na, D]
    out       = [feats_a, fused]                                [Na, 2D]

v3: single flat software pipeline per core (2 batches), emission-ordered
so every engine runs back-to-back:
  * PE: 64 "rounds", each = 1 distance half-tile (2x512-col fp16 matmuls,
    packed -(dist2+m/2048) values) + 1 feature tile (Yb or Ya, 4 matmuls).
    Continuous occupancy keeps the PE at the 2.4GHz p-state (it drops to
    1.2GHz whenever the engine idles and needs ~3us busy to re-ramit).
  * DVE: max8 scans trail the dist matmuls; z-adds and k-sums for the MLP
    combine are interleaved between later scans.
  * Act: PSUM->SBUF fp16 copies of Ya/Yb + relu(z)*dw (activation scale).
  * Pool: Q7 dma_gather of Yb rows (2 tiles / 1024 idx per call), round-
    robined over 4 SWDGE queues so descriptor rings never force a drain.
  * top-4 via max8 on packed fp32 PSUM: value = -(dist2 + m/2048) gives
    bit-exact jax.lax.top_k ordering incl. index tie-breaks; indices are
    unpacked arithmetically (no max_index rescan).

Sharding: data-parallel over batch (16 batches -> 8 cores x 2).
"""

import sys

sys.path.insert(0, "/opt/trn_rl_repo")

import numpy as np

import concourse.bass as bass
import concourse.mybir as mybir
import concourse.tile as tile
from concourse import bacc
from concourse.library_config import mlp as mlp_lib

F32 = mybir.dt.float32
F16 = mybir.dt.float16
U32 = mybir.dt.uint32
I16 = mybir.dt.int16

P = 128          # partitions
D = 512          # feature dim
KNN = 4          # neighbors
R = 0.5
FULL_SCALE = 128.0

B = 16           # full batch
N_CORES = 8
BLOC = B // N_CORES  # batches per core

NA = 2048
NB = 2048
NT = NA // P     # n-tiles per batch
DT = D // P      # 128-chunks of the feature dim
HALF = 1024      # distance column chunk (2 PSUM banks)

AF = mybir.ActivationFunctionType
ALU = mybir.AluOpType

N_QUEUES = 4     # SWDGE queues for dma_gather round-robin
GW = 2           # tiles per dma_gather call

# idx-build groups per batch: (start_tile, num_tiles). First b0 group is
# small so the first gather can be issued early (Pool is the critical
# engine; it must start as soon as possible).
GROUPS = {0: [(0, 8), (8, 8)], 1: [(0, 8), (8, 8)]}


def build_bass(bloc=BLOC, na=NA, nb=NB, with_bias=False):
    nc = bacc.Bacc("TRN2", debug=False, num_swdge_queues=N_QUEUES)
    fatT = nc.dram_tensor("fatT", [bloc, DT, P, na], F16, kind="ExternalInput").ap()
    fbtT = nc.dram_tensor("fbtT", [bloc, DT, P, nb], F16, kind="ExternalInput").ap()
    phia = nc.dram_tensor("phia", [bloc, 8, na], F16, kind="ExternalInput").ap()
    phib = nc.dram_tensor("phib", [bloc, 8, nb], F16, kind="ExternalInput").ap()
    w2 = nc.dram_tensor("w2", [DT, P, D], F16, kind="ExternalInput").ap()
    wd = nc.dram_tensor("wd", [DT, P, D], F16, kind="ExternalInput").ap()
    biasw = nc.dram_tensor("biasw", [1, D], F16, kind="ExternalInput").ap()
    ident = nc.dram_tensor("ident", [P, P], F16, kind="ExternalInput").ap()
    rep = nc.dram_tensor("rep", [16, P], F16, kind="ExternalInput").ap()
    out = nc.dram_tensor("out", [bloc, na, D], F16, kind="ExternalOutput").ap()

    nc.gpsimd.load_library(mlp_lib)
    with tile.TileContext(nc) as tc:
        _kern(tc, fatT, fbtT, phia, phib, w2, wd, biasw, ident, rep, out,
              bloc=bloc, na=na, nb=nb, with_bias=with_bias)
    nc.compile()
    return nc


def _kern(tc, fatT, fbtT, phia, phib, w2, wd, biasw, ident, rep, out, *,
          bloc, na, nb, with_bias):
    nc = tc.nc
    nt = na // P

    with (
        tc.tile_pool(name="const", bufs=1) as cpool,
        tc.tile_pool(name="wpool", bufs=1) as wpool,
        tc.tile_pool(name="feat", bufs=1) as fpool,
        tc.tile_pool(name="phi", bufs=1) as phipool,
        tc.tile_pool(name="tk", bufs=1) as tkpool,
        tc.tile_pool(name="ext", bufs=2) as epool,
        tc.tile_pool(name="h16p", bufs=3) as hpool,
        tc.tile_pool(name="io", bufs=3) as iopool,
        tc.tile_pool(name="ya", bufs=1) as yapool,
        tc.tile_pool(name="gat", bufs=3) as gpool,
        tc.tile_pool(name="mlp", bufs=2) as mpool,
        tc.tile_pool(name="fusedp", bufs=10) as fpool2,
        tc.tile_pool(name="dram", bufs=1, space="DRAM") as dpool,
        tc.tile_pool(name="dist_ps", bufs=2, space="PSUM") as dps,
        tc.tile_pool(name="mm_ps", bufs=3, space="PSUM") as mmps,
        tc.tile_pool(name="tp_ps", bufs=1, space="PSUM") as tpps,
    ):
        rconst = cpool.tile([P, 1], F32, name="rconst")
        nc.vector.memset(rconst, R)
        ones_t = cpool.tile([1, P], F16, name="ones_t")
        nc.vector.memset(ones_t, 1.0)

        # ---- upfront loads, ordered by first use on the critical path:
        # phia/phib gate the first dist matmul; wd + the first fbt-b0 slice
        # gate the first Yb matmul (Yb-b0 completion gates the first gather).
        phia_sb, phib_sb, fat_sb, fbt_sb = {}, {}, {}, {}
        for b in range(bloc):
            phia_sb[b] = phipool.tile([8, na], F16, tag=f"phia{b}",
                                      name=f"phia_sb{b}")
            phib_sb[b] = phipool.tile([8, nb], F16, tag=f"phib{b}",
                                      name=f"phib_sb{b}")
            nc.sync.dma_start(out=phia_sb[b], in_=phia[b])
            nc.sync.dma_start(out=phib_sb[b], in_=phib[b])
        w2_sb = wpool.tile([P, DT, D], F16, name="w2_sb")
        wd_sb = wpool.tile([P, DT, D], F16, name="wd_sb")
        for j in range(DT):
            nc.sync.dma_start(out=wd_sb[:, j, :], in_=wd[j])
        CS = nb // 4
        for b in range(bloc):
            fat_sb[b] = fpool.tile([P, DT, na], F16, tag=f"fat{b}",
                                   name=f"fat_sb{b}")
            fbt_sb[b] = fpool.tile([P, DT, nb], F16, tag=f"fbt{b}",
                                   name=f"fbt_sb{b}")
        for cslice in range(4):
            c0 = cslice * CS
            nc.sync.dma_start(
                out=fbt_sb[0][:, :, c0:c0 + CS],
                in_=fbtT[0, :, :, c0:c0 + CS].rearrange("j p c -> p j c"))
        for j in range(DT):
            nc.sync.dma_start(out=w2_sb[:, j, :], in_=w2[j])
        ident_sb = wpool.tile([P, P], F16, name="ident_sb")
        rep_sb = wpool.tile([16, P], F16, name="rep_sb")
        nc.sync.dma_start(out=ident_sb, in_=ident)
        nc.sync.dma_start(out=rep_sb, in_=rep)
        bias_sb = wpool.tile([1, D], F16, name="bias_sb")
        if with_bias:
            nc.sync.dma_start(out=bias_sb, in_=biasw)
        for cslice in range(4):
            c0 = cslice * CS
            nc.sync.dma_start(
                out=fat_sb[0][:, :, c0:c0 + CS],
                in_=fatT[0, :, :, c0:c0 + CS].rearrange("j p c -> p j c"))
        for b in range(1, bloc):
            for cslice in range(4):
                c0 = cslice * CS
                nc.sync.dma_start(
                    out=fbt_sb[b][:, :, c0:c0 + CS],
                    in_=fbtT[b, :, :, c0:c0 + CS].rearrange("j p c -> p j c"))
                nc.sync.dma_start(
                    out=fat_sb[b][:, :, c0:c0 + CS],
                    in_=fatT[b, :, :, c0:c0 + CS].rearrange("j p c -> p j c"))

        # ---- per-batch state tiles ----
        yb_dram, negd, t_all, dw, ya_all = {}, {}, {}, {}, {}
        for b in range(bloc):
            yb_dram[b] = dpool.tile([nb, D], F16, tag=f"ybd{b}",
                                    name=f"yb_dram{b}")
            negd[b] = tkpool.tile([P, nt, 8], F32, tag=f"negd{b}",
                                  name=f"negd{b}")
            t_all[b] = tkpool.tile([P, nt, 32], I16, tag=f"tall{b}",
                                   name=f"t_all{b}")
            dw[b] = tkpool.tile([P, nt, KNN], F32, tag=f"dw{b}",
                                name=f"dw{b}")
            ya_all[b] = yapool.tile([P, nt, D], F16, tag=f"ya{b}",
                                    name=f"ya_all{b}")

        # ---------- emission helpers ----------
        live_dist = {}
        live_h16 = {}

        def emit_dist(b, i, h):
            ps = dps.tile([P, HALF], F32, tag="dist", name="dist_ps")
            live_dist[(b, i, h)] = ps
            for q in range(2):
                c0 = h * HALF + q * 512
                nc.tensor.matmul(
                    out=ps[:, q * 512:(q + 1) * 512],
                    lhsT=phia_sb[b][:, i * P:(i + 1) * P],
                    rhs=phib_sb[b][:, c0:c0 + 512],
                    start=True, stop=True)

        def emit_scan(b, i, h):
            if h == 0:
                live_h16[(b, i)] = hpool.tile([P, 16], F32, tag="h16",
                                              name="h16")
            h16 = live_h16[(b, i)]
            nc.vector.max(out=h16[:, h * 8:(h + 1) * 8],
                          in_=live_dist.pop((b, i, h)))
            if h == 1:
                nc.vector.max(out=negd[b][:, i, :],
                              in_=live_h16.pop((b, i)))

        def emit_feat(b, kind, i):
            ps = mmps.tile([P, D], F32, tag="mm", name=f"{kind}_ps")
            src = fbt_sb[b] if kind == "yb" else fat_sb[b]
            wmat = wd_sb if kind == "yb" else w2_sb
            for j in range(DT):
                nc.tensor.matmul(
                    out=ps, lhsT=src[:, j, i * P:(i + 1) * P],
                    rhs=wmat[:, j, :],
                    start=(j == 0),
                    stop=(j == DT - 1 and not (kind == "ya" and with_bias)))
            if kind == "ya" and with_bias:
                nc.tensor.matmul(out=ps, lhsT=ones_t, rhs=bias_sb,
                                 start=False, stop=True)
            if kind == "yb":
                yb_sb = iopool.tile([P, D], F16, tag="ybsb", name="yb_sb")
                nc.scalar.copy(out=yb_sb, in_=ps)
                nc.sync.dma_start(out=yb_dram[b][i * P:(i + 1) * P, :],
                                  in_=yb_sb)
            else:
                nc.scalar.copy(out=ya_all[b][:, i, :], in_=ps)

        def emit_build(b, g0, glen):
            """Unpack idx/dw for tiles [g0, g0+glen) and build the wrapped
            int16 gather index list in t_all[b]."""
            nsl = slice(g0, g0 + glen)
            # packed = -(dist2 + m/2048); y = 2048*dist2 + m (exact int);
            # idx = y & 2047; dw = relu(R - sqrt((y - idx) * 2^-25))
            y_f = epool.tile([P, glen, KNN], F32, tag="y_f", name="y_f")
            nc.vector.tensor_scalar(
                out=y_f, in0=negd[b][:, nsl, 0:KNN], scalar1=-2048.0,
                scalar2=None, op0=ALU.mult)
            y_u = epool.tile([P, glen, KNN], U32, tag="y_u", name="y_u")
            nc.vector.tensor_copy(out=y_u, in_=y_f)
            idx_u = epool.tile([P, glen, KNN], U32, tag="idx_u", name="idx_u")
            nc.vector.tensor_scalar(
                out=idx_u, in0=y_u, scalar1=2047, scalar2=None,
                op0=ALU.bitwise_and)
            d2_f = epool.tile([P, glen, KNN], F32, tag="d2_f", name="d2_f")
            nc.vector.tensor_tensor(out=d2_f, in0=y_u, in1=idx_u,
                                    op=ALU.subtract)
            dist4 = epool.tile([P, glen, KNN], F32, tag="dist4", name="dist4")
            nc.scalar.activation(out=dist4, in_=d2_f, func=AF.Sqrt,
                                 scale=float(2.0 ** -25))
            nc.scalar.activation(out=dw[b][:, nsl, :], in_=dist4, func=AF.Relu,
                                 scale=-1.0, bias=rconst)

            # wrapped idx list: t_all[16g+c, i, 8k+pp] = idx[16pp+c, i, k]
            # for every group g (PE transposes + one-hot replicate matmul).
            nq = glen * KNN
            idxf = epool.tile([P, nq], F16, tag="idxf", name="idxf")
            nc.vector.tensor_copy(
                out=idxf, in_=idx_u[:].rearrange("p i k -> p (i k)"))
            tpx0 = tpps.tile([nt * KNN, P], F16, tag="tpx", name="idxT_ps")
            idxT_ps = tpx0[0:nq, :]
            nc.tensor.transpose(out=idxT_ps, in_=idxf, identity=ident_sb)
            idxT_sb = epool.tile([nt * KNN, P], F16, tag="idxT_sb",
                                 name="idxT_sb")
            nc.scalar.copy(out=idxT_sb[0:nq, :], in_=idxT_ps)
            t16 = epool.tile([16, nt, 32], F16, tag="t16", name="t16")
            for pp in range(8):
                tp_big = tpps.tile([nt * KNN, P], F16, tag="tpx",
                                   name="tp_ps")
                tp_ps = tp_big[0:16, 0:nq]
                nc.tensor.transpose(
                    out=tp_ps, in_=idxT_sb[0:nq, 16 * pp:16 * (pp + 1)],
                    identity=ident_sb[0:nq, 0:nq])
                nc.scalar.copy(
                    out=t16[:, 0:glen, pp::8],
                    in_=tp_ps.rearrange("c (i k) -> c i k", i=glen, k=KNN))
            rep_full = mmps.tile([P, D], F32, tag="mm", name="rep_ps")
            rep_ps = rep_full[:, 0:glen * 32]
            nc.tensor.matmul(
                out=rep_ps, lhsT=rep_sb,
                rhs=t16[:, 0:glen, :].rearrange("c i j -> c (i j)"),
                start=True, stop=True)
            nc.scalar.copy(
                out=t_all[b][:, nsl, :].rearrange("p i j -> p (i j)"),
                in_=rep_ps)

        live_gather = {}
        gather_count = [0]

        def emit_gather(b, ig):
            ybg = gpool.tile([P, GW, KNN, D], F16, tag="ybg", name="ybg")
            live_gather[(b, ig)] = ybg
            qn = gather_count[0] % N_QUEUES
            gather_count[0] += 1
            nc.gpsimd.dma_gather(
                out_ap=ybg[:].rearrange("p g k d -> p (g k) d"),
                in_ap=yb_dram[b][:],
                idxs_ap=t_all[b][:, ig:ig + GW, :].rearrange(
                    "p g j -> p (g j)"),
                num_idxs=P * KNN * GW, num_idxs_reg=P * KNN * GW,
                elem_size=D, queue_num=qn)

        live_r = {}

        def emit_combine_z(b, i):
            ig = (i // GW) * GW
            ybgi = live_gather[(b, ig)][:, i - ig]
            if i - ig == GW - 1:
                del live_gather[(b, ig)]
            z = mpool.tile([P, KNN, D], F16, tag="z", name="z")
            ya_b = ya_all[b][:, i, :].unsqueeze(1).broadcast_to([P, KNN, D])
            nc.vector.tensor_tensor(out=z, in0=ybgi, in1=ya_b, op=ALU.add)
            r = mpool.tile([P, KNN, D], F16, tag="r", name="r")
            live_r[(b, i)] = r
            for k in range(KNN):
                nc.scalar.activation(
                    out=r[:, k, :], in_=z[:, k, :],
                    func=AF.Relu, scale=dw[b][:, i, k:k + 1])

        live_fused = {}

        def emit_combine_s(b, i):
            r = live_r.pop((b, i))
            s01 = mpool.tile([P, 2, D], F16, tag="s01", name="s01")
            nc.vector.tensor_tensor(out=s01, in0=r[:, 0:2, :],
                                    in1=r[:, 2:4, :], op=ALU.add)
            fused = fpool2.tile([P, D], F16, tag="fused", name="fused")
            live_fused[(b, i)] = fused
            nc.vector.tensor_add(fused, s01[:, 0, :], s01[:, 1, :])

        def emit_out(b, i):
            # out-DMAs issue on the (FIFO) sync queue strictly after every
            # yb_dram write: a not-yet-ready fused tile at the queue head
            # would otherwise stall the yb writes behind it, which delays the
            # first gather and cascades through the whole pipeline.
            nc.sync.dma_start(out=out[b, i * P:(i + 1) * P, :],
                              in_=live_fused.pop((b, i)))

        # ---------- static schedule ----------
        # dist halves in batch-major order; feature tiles: yb b0, ya b0,
        # yb b1, ya b1.
        dist_halves = [(b, i, h) for b in range(bloc) for i in range(nt)
                       for h in range(2)]
        feats = []
        for b in range(bloc):
            feats += [(b, "yb", i) for i in range(nt)]
            feats += [(b, "ya", i) for i in range(nt)]
        # builds due after the last scan round of their tile group
        build_at = {}
        for b in range(bloc):
            for (g0, glen) in GROUPS[b]:
                last_round = b * 2 * nt + (g0 + glen) * 2  # rounds consumed
                build_at.setdefault(last_round, []).append((b, g0, glen))

        # gather calls in issue order; each covers GW tiles. Combines must
        # trail their gather closely in emission order (the 3-deep ybg ring
        # means gather g+3 reuses g's slot; a reader emitted later than that
        # writer is an unenforced WAR hazard -> data corruption). Each
        # combine is split into two ~1.2us DVE insertions so the scan stream
        # never stalls long enough to backpressure the PE off its p-state.
        gather_sched = []   # (round_due, b, ig)
        combine_sched = []  # (round_due, kind, b, i)
        for b in range(bloc):
            for (g0, glen) in GROUPS[b]:
                due = b * 2 * nt + (g0 + glen) * 2 + 1
                for j, ig in enumerate(range(g0, g0 + glen, GW)):
                    gather_sched.append((max(due, 24) + 3 * j, b, ig))
                    cd = max(due, 24) + 3 * j
                    combine_sched.append((cd + 4, 'z', b, ig))
                    combine_sched.append((cd + 5, 's', b, ig))
                    combine_sched.append((cd + 6, 'z', b, ig + 1))
                    combine_sched.append((cd + 7, 's', b, ig + 1))
        # out-DMAs: after the last yb_dram write round, in combine order
        last_yb_round = (bloc - 1) * 2 * nt + nt + 1
        out_sched = [(max(c[0] + 2, last_yb_round + jj), c[2], c[3])
                     for jj, c in enumerate(
                         [c for c in combine_sched if c[1] == 's'])]

        n_rounds = len(dist_halves)
        gi, ci, oi = 0, 0, 0
        for r in range(n_rounds + 60):
            if r < n_rounds:
                b, i, h = dist_halves[r]
                emit_dist(b, i, h)
                emit_feat(*feats[r])
                emit_scan(b, i, h)
            for (b, g0, glen) in build_at.get(r + 1, []):
                emit_build(b, g0, glen)
            while gi < len(gather_sched) and gather_sched[gi][0] <= r + 1:
                emit_gather(gather_sched[gi][1], gather_sched[gi][2])
                gi += 1
            while ci < len(combine_sched) and combine_sched[ci][0] <= r + 1:
                _, kind, cb, cti = combine_sched[ci]
                if kind == 'z':
                    emit_combine_z(cb, cti)
                else:
                    emit_combine_s(cb, cti)
                ci += 1
            while oi < len(out_sched) and out_sched[oi][0] <= r + 1:
                emit_out(out_sched[oi][1], out_sched[oi][2])
                oi += 1
        assert gi == len(gather_sched) and ci == len(combine_sched) \
            and oi == len(out_sched), (gi, ci, oi)


# ---------------------------------------------------------------------------
# host side
# ---------------------------------------------------------------------------

def _host_inputs(feats_a, feats_b, W, bias, coords_a, coords_b):
    """Host-side prep: fp16 casts, chunk transposes, lifted packed coords."""
    d = W.shape[1]
    dt = d // P
    bsz, na_, _ = feats_a.shape
    nb_ = feats_b.shape[1]

    ca = coords_a.astype(np.int64)
    cb = coords_b.astype(np.int64)
    a2 = (ca * ca).sum(-1)                      # [B, Na] ints < 48388
    b2 = (cb * cb).sum(-1)
    hiA, loA = a2 >> 11, a2 & 2047
    hiB, loB = b2 >> 11, b2 & 2047
    m_over = (np.arange(nb_, dtype=np.float32) / 2048.0)[None, :].repeat(bsz, 0)
    # packed dot = 2a.b - |a|^2 - |b|^2 - m/2048 = -(dist2 + m/2048)
    phia8 = np.stack([ca[..., 0], ca[..., 1], ca[..., 2], hiA, loA,
                      2048 * np.ones((bsz, na_), np.int64),
                      np.ones((bsz, na_), np.int64),
                      np.ones((bsz, na_), np.int64)], axis=1).astype(np.float16)
    phib8 = np.stack([2.0 * cb[..., 0], 2.0 * cb[..., 1], 2.0 * cb[..., 2],
                      -2048 * np.ones((bsz, nb_), np.float64),
                      -np.ones((bsz, nb_), np.float64),
                      -hiB.astype(np.float64), -loB.astype(np.float64),
                      -m_over.astype(np.float64)], axis=1).astype(np.float16)

    # feats chunk-transposed: [B, dt, 128, N]
    fatT = np.ascontiguousarray(
        feats_a.reshape(bsz, na_, dt, P).transpose(0, 2, 3, 1)).astype(np.float16)
    fbtT = np.ascontiguousarray(
        feats_b.reshape(bsz, nb_, dt, P).transpose(0, 2, 3, 1)).astype(np.float16)

    w2f = W[d:]                                  # applies to a_f
    wdf = W[:d] - W[d:]                          # applies to b_f
    w2c = np.ascontiguousarray(w2f.reshape(dt, P, d)).astype(np.float16)
    wdc = np.ascontiguousarray(wdf.reshape(dt, P, d)).astype(np.float16)
    biasw = bias.reshape(1, d).astype(np.float16)
    return fatT, fbtT, phia8, phib8, w2c, wdc, biasw


def kernel(**inputs):
    feats_a = np.asarray(inputs["feats_a"], dtype=np.float32)
    feats_b = np.asarray(inputs["feats_b"], dtype=np.float32)
    W = np.asarray(inputs["W"], dtype=np.float32)
    bias = np.asarray(inputs["bias"], dtype=np.float32)
    coords_a = np.asarray(inputs["coords_a"])
    coords_b = np.asarray(inputs["coords_b"])

    fatT, fbtT, phia8, phib8, w2c, wdc, biasw = _host_inputs(
        feats_a, feats_b, W, bias, coords_a, coords_b)
    with_bias = bool(np.any(bias != 0.0))
    identm = np.eye(P, dtype=np.float16)
    repm = np.zeros((16, P), np.float16)
    for p_ in range(P):
        repm[p_ % 16, p_] = 1.0

    nc = build_bass(with_bias=with_bias)

    in_maps = []
    for c in range(N_CORES):
        s = slice(c * BLOC, (c + 1) * BLOC)
        in_maps.append({
            "fatT": np.ascontiguousarray(fatT[s]),
            "fbtT": np.ascontiguousarray(fbtT[s]),
            "phia": np.ascontiguousarray(phia8[s]),
            "phib": np.ascontiguousarray(phib8[s]),
            "w2": w2c,
            "wd": wdc,
            "biasw": biasw,
            "ident": identm,
            "rep": repm,
        })

    from concourse import bass_utils
    res = bass_utils.run_bass_kernel_spmd(nc, in_maps, core_ids=list(range(N_CORES)))
    fused = np.concatenate([r["out"] for r in res.results], axis=0)
    return np.concatenate([feats_a, fused.astype(np.float32)], axis=-1)


if __name__ == "__main__":
    nc = build_bass()
    print("built ok")


# revision 26
# speedup vs baseline: 1.3092x; 1.0239x over previous
"""Trainium2 Bass kernel for retrieval_knn (nn_CLI_v1_63702954934484).

Reference computation (per batch b):
    dist[n,m] = ||ca[n] - cb[m]|| / 128                         [Na, Nb]
    idx       = argtop4-smallest(dist[n,:])                     [Na, 4]
    dw        = R - clip(dist_top4, 0, R)                       [Na, 4]
    h         = [b_f, a_f - b_f]  (b_f = feats_b[idx])          [Na, 4, 2D]
    fused     = sum_k relu(h @ W + bias) * dw                   [Na, D]
    out       = [feats_a, fused]                                [Na, 2D]

v3: single flat software pipeline per core (2 batches), emission-ordered
so every engine runs back-to-back:
  * PE: 64 "rounds", each = 1 distance half-tile (2x512-col fp16 matmuls,
    packed -(dist2+m/2048) values) + 1 feature tile (Yb or Ya, 4 matmuls).
    Continuous occupancy keeps the PE at the 2.4GHz p-state (it drops to
    1.2GHz whenever the engine idles and needs ~3us busy to re-ramit).
  * DVE: max8 scans trail the dist matmuls; z-adds and k-sums for the MLP
    combine are interleaved between later scans.
  * Act: PSUM->SBUF fp16 copies of Ya/Yb + relu(z)*dw (activation scale).
  * Pool: Q7 dma_gather of Yb rows (2 tiles / 1024 idx per call), round-
    robined over 4 SWDGE queues so descriptor rings never force a drain.
  * top-4 via max8 on packed fp32 PSUM: value = -(dist2 + m/2048) gives
    bit-exact jax.lax.top_k ordering incl. index tie-breaks; indices are
    unpacked arithmetically (no max_index rescan).

Sharding: data-parallel over batch (16 batches -> 8 cores x 2).
"""

import sys

sys.path.insert(0, "/opt/trn_rl_repo")

import numpy as np

import concourse.bass as bass
import concourse.mybir as mybir
import concourse.tile as tile
from concourse import bacc
from concourse.library_config import mlp as mlp_lib

F32 = mybir.dt.float32
F16 = mybir.dt.float16
U32 = mybir.dt.uint32
I16 = mybir.dt.int16

P = 128          # partitions
D = 512          # feature dim
KNN = 4          # neighbors
R = 0.5
FULL_SCALE = 128.0

B = 16           # full batch
N_CORES = 8
BLOC = B // N_CORES  # batches per core

NA = 2048
NB = 2048
NT = NA // P     # n-tiles per batch
DT = D // P      # 128-chunks of the feature dim
HALF = 1024      # distance column chunk (2 PSUM banks)

AF = mybir.ActivationFunctionType
ALU = mybir.AluOpType

N_QUEUES = 4     # SWDGE queues for dma_gather round-robin
GW = 2           # tiles per dma_gather call

# idx-build groups per batch: (start_tile, num_tiles). First b0 group is
# small so the first gather can be issued early (Pool is the critical
# engine; it must start as soon as possible).
GROUPS = {0: [(0, 8), (8, 8)], 1: [(0, 8), (8, 4), (12, 4)]}


def build_bass(bloc=BLOC, na=NA, nb=NB, with_bias=False):
    nc = bacc.Bacc("TRN2", debug=False, num_swdge_queues=N_QUEUES)
    fatT = nc.dram_tensor("fatT", [bloc, DT, P, na], F16, kind="ExternalInput").ap()
    fbtT = nc.dram_tensor("fbtT", [bloc, DT, P, nb], F16, kind="ExternalInput").ap()
    phia = nc.dram_tensor("phia", [bloc, 8, na], F16, kind="ExternalInput").ap()
    phib = nc.dram_tensor("phib", [bloc, 8, nb], F16, kind="ExternalInput").ap()
    w2 = nc.dram_tensor("w2", [DT, P, D], F16, kind="ExternalInput").ap()
    wd = nc.dram_tensor("wd", [DT, P, D], F16, kind="ExternalInput").ap()
    biasw = nc.dram_tensor("biasw", [1, D], F16, kind="ExternalInput").ap()
    ident = nc.dram_tensor("ident", [P, P], F16, kind="ExternalInput").ap()
    rep = nc.dram_tensor("rep", [16, P], F16, kind="ExternalInput").ap()
    out = nc.dram_tensor("out", [bloc, na, D], F16, kind="ExternalOutput").ap()

    nc.gpsimd.load_library(mlp_lib)
    with tile.TileContext(nc) as tc:
        _kern(tc, fatT, fbtT, phia, phib, w2, wd, biasw, ident, rep, out,
              bloc=bloc, na=na, nb=nb, with_bias=with_bias)
    nc.compile()
    return nc


def _kern(tc, fatT, fbtT, phia, phib, w2, wd, biasw, ident, rep, out, *,
          bloc, na, nb, with_bias):
    nc = tc.nc
    nt = na // P

    with (
        tc.tile_pool(name="const", bufs=1) as cpool,
        tc.tile_pool(name="wpool", bufs=1) as wpool,
        tc.tile_pool(name="feat", bufs=1) as fpool,
        tc.tile_pool(name="phi", bufs=1) as phipool,
        tc.tile_pool(name="tk", bufs=1) as tkpool,
        tc.tile_pool(name="ext", bufs=2) as epool,
        tc.tile_pool(name="h16p", bufs=3) as hpool,
        tc.tile_pool(name="io", bufs=3) as iopool,
        tc.tile_pool(name="ya", bufs=1) as yapool,
        tc.tile_pool(name="gat", bufs=4) as gpool,
        tc.tile_pool(name="mlp", bufs=2) as mpool,
        tc.tile_pool(name="fusedp", bufs=10) as fpool2,
        tc.tile_pool(name="dram", bufs=1, space="DRAM") as dpool,
        tc.tile_pool(name="dist_ps", bufs=2, space="PSUM") as dps,
        tc.tile_pool(name="mm_ps", bufs=3, space="PSUM") as mmps,
        tc.tile_pool(name="tp_ps", bufs=1, space="PSUM") as tpps,
    ):
        rconst = cpool.tile([P, 1], F32, name="rconst")
        nc.vector.memset(rconst, R)
        ones_t = cpool.tile([1, P], F16, name="ones_t")
        nc.vector.memset(ones_t, 1.0)

        # ---- upfront loads, ordered by first use on the critical path:
        # phia/phib gate the first dist matmul; wd + the first fbt-b0 slice
        # gate the first Yb matmul (Yb-b0 completion gates the first gather).
        phia_sb, phib_sb, fat_sb, fbt_sb = {}, {}, {}, {}
        for b in range(bloc):
            phia_sb[b] = phipool.tile([8, na], F16, tag=f"phia{b}",
                                      name=f"phia_sb{b}")
            phib_sb[b] = phipool.tile([8, nb], F16, tag=f"phib{b}",
                                      name=f"phib_sb{b}")
            nc.sync.dma_start(out=phia_sb[b], in_=phia[b])
            nc.sync.dma_start(out=phib_sb[b], in_=phib[b])
        w2_sb = wpool.tile([P, DT, D], F16, name="w2_sb")
        wd_sb = wpool.tile([P, DT, D], F16, name="wd_sb")
        for j in range(DT):
            nc.sync.dma_start(out=wd_sb[:, j, :], in_=wd[j])
        CS = nb // 4
        for b in range(bloc):
            fat_sb[b] = fpool.tile([P, DT, na], F16, tag=f"fat{b}",
                                   name=f"fat_sb{b}")
            fbt_sb[b] = fpool.tile([P, DT, nb], F16, tag=f"fbt{b}",
                                   name=f"fbt_sb{b}")
        for cslice in range(4):
            c0 = cslice * CS
            nc.sync.dma_start(
                out=fbt_sb[0][:, :, c0:c0 + CS],
                in_=fbtT[0, :, :, c0:c0 + CS].rearrange("j p c -> p j c"))
        for j in range(DT):
            nc.sync.dma_start(out=w2_sb[:, j, :], in_=w2[j])
        ident_sb = wpool.tile([P, P], F16, name="ident_sb")
        rep_sb = wpool.tile([16, P], F16, name="rep_sb")
        nc.sync.dma_start(out=ident_sb, in_=ident)
        nc.sync.dma_start(out=rep_sb, in_=rep)
        bias_sb = wpool.tile([1, D], F16, name="bias_sb")
        if with_bias:
            nc.sync.dma_start(out=bias_sb, in_=biasw)
        for cslice in range(4):
            c0 = cslice * CS
            nc.sync.dma_start(
                out=fat_sb[0][:, :, c0:c0 + CS],
                in_=fatT[0, :, :, c0:c0 + CS].rearrange("j p c -> p j c"))
        for b in range(1, bloc):
            for cslice in range(4):
                c0 = cslice * CS
                nc.sync.dma_start(
                    out=fbt_sb[b][:, :, c0:c0 + CS],
                    in_=fbtT[b, :, :, c0:c0 + CS].rearrange("j p c -> p j c"))
                nc.sync.dma_start(
                    out=fat_sb[b][:, :, c0:c0 + CS],
                    in_=fatT[b, :, :, c0:c0 + CS].rearrange("j p c -> p j c"))

        # ---- per-batch state tiles ----
        yb_dram, negd, t_all, dw, ya_all = {}, {}, {}, {}, {}
        for b in range(bloc):
            yb_dram[b] = dpool.tile([nb, D], F16, tag=f"ybd{b}",
                                    name=f"yb_dram{b}")
            negd[b] = tkpool.tile([P, nt, 8], F32, tag=f"negd{b}",
                                  name=f"negd{b}")
            t_all[b] = tkpool.tile([P, nt, 32], I16, tag=f"tall{b}",
                                   name=f"t_all{b}")
            dw[b] = tkpool.tile([P, nt, KNN], F32, tag=f"dw{b}",
                                name=f"dw{b}")
            ya_all[b] = yapool.tile([P, nt, D], F16, tag=f"ya{b}",
                                    name=f"ya_all{b}")

        # ---------- emission helpers ----------
        live_dist = {}
        live_h16 = {}

        def emit_dist(b, i, h):
            ps = dps.tile([P, HALF], F32, tag="dist", name="dist_ps")
            live_dist[(b, i, h)] = ps
            for q in range(2):
                c0 = h * HALF + q * 512
                nc.tensor.matmul(
                    out=ps[:, q * 512:(q + 1) * 512],
                    lhsT=phia_sb[b][:, i * P:(i + 1) * P],
                    rhs=phib_sb[b][:, c0:c0 + 512],
                    start=True, stop=True)

        def emit_scan(b, i, h):
            if h == 0:
                live_h16[(b, i)] = hpool.tile([P, 16], F32, tag="h16",
                                              name="h16")
            h16 = live_h16[(b, i)]
            nc.vector.max(out=h16[:, h * 8:(h + 1) * 8],
                          in_=live_dist.pop((b, i, h)))
            if h == 1:
                nc.vector.max(out=negd[b][:, i, :],
                              in_=live_h16.pop((b, i)))

        def emit_feat(b, kind, i):
            ps = mmps.tile([P, D], F32, tag="mm", name=f"{kind}_ps")
            src = fbt_sb[b] if kind == "yb" else fat_sb[b]
            wmat = wd_sb if kind == "yb" else w2_sb
            for j in range(DT):
                nc.tensor.matmul(
                    out=ps, lhsT=src[:, j, i * P:(i + 1) * P],
                    rhs=wmat[:, j, :],
                    start=(j == 0),
                    stop=(j == DT - 1 and not (kind == "ya" and with_bias)))
            if kind == "ya" and with_bias:
                nc.tensor.matmul(out=ps, lhsT=ones_t, rhs=bias_sb,
                                 start=False, stop=True)
            if kind == "yb":
                yb_sb = iopool.tile([P, D], F16, tag="ybsb", name="yb_sb")
                nc.scalar.copy(out=yb_sb, in_=ps)
                nc.sync.dma_start(out=yb_dram[b][i * P:(i + 1) * P, :],
                                  in_=yb_sb)
            else:
                nc.scalar.copy(out=ya_all[b][:, i, :], in_=ps)

        def emit_build(b, g0, glen):
            """Unpack idx/dw for tiles [g0, g0+glen) and build the wrapped
            int16 gather index list in t_all[b]."""
            nsl = slice(g0, g0 + glen)
            # packed = -(dist2 + m/2048); y = 2048*dist2 + m (exact int);
            # idx = y & 2047; dw = relu(R - sqrt((y - idx) * 2^-25))
            y_f = epool.tile([P, glen, KNN], F32, tag="y_f", name="y_f")
            nc.vector.tensor_scalar(
                out=y_f, in0=negd[b][:, nsl, 0:KNN], scalar1=-2048.0,
                scalar2=None, op0=ALU.mult)
            y_u = epool.tile([P, glen, KNN], U32, tag="y_u", name="y_u")
            nc.vector.tensor_copy(out=y_u, in_=y_f)
            idx_u = epool.tile([P, glen, KNN], U32, tag="idx_u", name="idx_u")
            nc.vector.tensor_scalar(
                out=idx_u, in0=y_u, scalar1=2047, scalar2=None,
                op0=ALU.bitwise_and)
            d2_f = epool.tile([P, glen, KNN], F32, tag="d2_f", name="d2_f")
            nc.vector.tensor_tensor(out=d2_f, in0=y_u, in1=idx_u,
                                    op=ALU.subtract)
            dist4 = epool.tile([P, glen, KNN], F32, tag="dist4", name="dist4")
            nc.scalar.activation(out=dist4, in_=d2_f, func=AF.Sqrt,
                                 scale=float(2.0 ** -25))
            nc.scalar.activation(out=dw[b][:, nsl, :], in_=dist4, func=AF.Relu,
                                 scale=-1.0, bias=rconst)

            # wrapped idx list: t_all[16g+c, i, 8k+pp] = idx[16pp+c, i, k]
            # for every group g (PE transposes + one-hot replicate matmul).
            nq = glen * KNN
            idxf = epool.tile([P, nq], F16, tag="idxf", name="idxf")
            nc.vector.tensor_copy(
                out=idxf, in_=idx_u[:].rearrange("p i k -> p (i k)"))
            tpx0 = tpps.tile([nt * KNN, P], F16, tag="tpx", name="idxT_ps")
            idxT_ps = tpx0[0:nq, :]
            nc.tensor.transpose(out=idxT_ps, in_=idxf, identity=ident_sb)
            idxT_sb = epool.tile([nt * KNN, P], F16, tag="idxT_sb",
                                 name="idxT_sb")
            nc.scalar.copy(out=idxT_sb[0:nq, :], in_=idxT_ps)
            t16 = epool.tile([16, nt, 32], F16, tag="t16", name="t16")
            for pp in range(8):
                tp_big = tpps.tile([nt * KNN, P], F16, tag="tpx",
                                   name="tp_ps")
                tp_ps = tp_big[0:16, 0:nq]
                nc.tensor.transpose(
                    out=tp_ps, in_=idxT_sb[0:nq, 16 * pp:16 * (pp + 1)],
                    identity=ident_sb[0:nq, 0:nq])
                nc.scalar.copy(
                    out=t16[:, 0:glen, pp::8],
                    in_=tp_ps.rearrange("c (i k) -> c i k", i=glen, k=KNN))
            rep_full = mmps.tile([P, D], F32, tag="mm", name="rep_ps")
            rep_ps = rep_full[:, 0:glen * 32]
            nc.tensor.matmul(
                out=rep_ps, lhsT=rep_sb,
                rhs=t16[:, 0:glen, :].rearrange("c i j -> c (i j)"),
                start=True, stop=True)
            nc.scalar.copy(
                out=t_all[b][:, nsl, :].rearrange("p i j -> p (i j)"),
                in_=rep_ps)

        live_gather = {}
        gather_count = [0]

        def emit_gather(b, ig):
            ybg = gpool.tile([P, GW, KNN, D], F16, tag="ybg", name="ybg")
            live_gather[(b, ig)] = ybg
            qn = gather_count[0] % N_QUEUES
            gather_count[0] += 1
            nc.gpsimd.dma_gather(
                out_ap=ybg[:].rearrange("p g k d -> p (g k) d"),
                in_ap=yb_dram[b][:],
                idxs_ap=t_all[b][:, ig:ig + GW, :].rearrange(
                    "p g j -> p (g j)"),
                num_idxs=P * KNN * GW, num_idxs_reg=P * KNN * GW,
                elem_size=D, queue_num=qn)

        live_r = {}

        def emit_combine_z(b, i):
            ig = (i // GW) * GW
            ybgi = live_gather[(b, ig)][:, i - ig]
            if i - ig == GW - 1:
                del live_gather[(b, ig)]
            z = mpool.tile([P, KNN, D], F16, tag="z", name="z")
            ya_b = ya_all[b][:, i, :].unsqueeze(1).broadcast_to([P, KNN, D])
            nc.vector.tensor_tensor(out=z, in0=ybgi, in1=ya_b, op=ALU.add)
            r = mpool.tile([P, KNN, D], F16, tag="r", name="r")
            live_r[(b, i)] = r
            for k in range(KNN):
                nc.scalar.activation(
                    out=r[:, k, :], in_=z[:, k, :],
                    func=AF.Relu, scale=dw[b][:, i, k:k + 1])

        live_fused = {}

        def emit_combine_s(b, i):
            r = live_r.pop((b, i))
            s01 = mpool.tile([P, 2, D], F16, tag="s01", name="s01")
            nc.vector.tensor_tensor(out=s01, in0=r[:, 0:2, :],
                                    in1=r[:, 2:4, :], op=ALU.add)
            fused = fpool2.tile([P, D], F16, tag="fused", name="fused")
            live_fused[(b, i)] = fused
            nc.vector.tensor_add(fused, s01[:, 0, :], s01[:, 1, :])

        def emit_out(b, i):
            # out-DMAs issue on the (FIFO) sync queue strictly after every
            # yb_dram write: a not-yet-ready fused tile at the queue head
            # would otherwise stall the yb writes behind it, which delays the
            # first gather and cascades through the whole pipeline.
            nc.sync.dma_start(out=out[b, i * P:(i + 1) * P, :],
                              in_=live_fused.pop((b, i)))

        # ---------- static schedule ----------
        # dist halves in batch-major order; feature tiles: yb b0, ya b0,
        # yb b1, ya b1.
        dist_halves = [(b, i, h) for b in range(bloc) for i in range(nt)
                       for h in range(2)]
        feats = []
        for b in range(bloc):
            feats += [(b, "yb", i) for i in range(nt)]
            feats += [(b, "ya", i) for i in range(nt)]
        # builds due after the last scan round of their tile group
        build_at = {}
        for b in range(bloc):
            for (g0, glen) in GROUPS[b]:
                last_round = b * 2 * nt + (g0 + glen) * 2  # rounds consumed
                build_at.setdefault(last_round, []).append((b, g0, glen))

        # gather calls in issue order; each covers GW tiles. Combines must
        # trail their gather closely in emission order (the 3-deep ybg ring
        # means gather g+3 reuses g's slot; a reader emitted later than that
        # writer is an unenforced WAR hazard -> data corruption). Each
        # combine is split into two ~1.2us DVE insertions so the scan stream
        # never stalls long enough to backpressure the PE off its p-state.
        gather_sched = []   # (round_due, b, ig)
        combine_sched = []  # (round_due, kind, b, i)
        for b in range(bloc):
            for (g0, glen) in GROUPS[b]:
                due = b * 2 * nt + (g0 + glen) * 2 + 1
                for j, ig in enumerate(range(g0, g0 + glen, GW)):
                    gather_sched.append((max(due, 24) + 3 * j, b, ig))
                    cd = max(due, 24) + 3 * j
                    # combines trail their gather by ~7 rounds: late enough
                    # that DVE keeps draining scans (which gate builds ->
                    # gathers -> Pool, the critical engine), early enough to
                    # stay within the 4-deep ybg ring's WAR distance.
                    combine_sched.append((cd + 7, 'z', b, ig))
                    combine_sched.append((cd + 8, 's', b, ig))
                    combine_sched.append((cd + 9, 'z', b, ig + 1))
                    combine_sched.append((cd + 10, 's', b, ig + 1))
        # out-DMAs: after the last yb_dram write round, in combine order
        last_yb_round = (bloc - 1) * 2 * nt + nt + 1
        out_sched = [(max(c[0] + 2, last_yb_round + jj), c[2], c[3])
                     for jj, c in enumerate(
                         [c for c in combine_sched if c[1] == 's'])]

        n_rounds = len(dist_halves)
        gi, ci, oi = 0, 0, 0
        for r in range(n_rounds + 60):
            if r < n_rounds:
                b, i, h = dist_halves[r]
                emit_dist(b, i, h)
                emit_feat(*feats[r])
                emit_scan(b, i, h)
            for (b, g0, glen) in build_at.get(r + 1, []):
                emit_build(b, g0, glen)
            while gi < len(gather_sched) and gather_sched[gi][0] <= r + 1:
                emit_gather(gather_sched[gi][1], gather_sched[gi][2])
                gi += 1
            while ci < len(combine_sched) and combine_sched[ci][0] <= r + 1:
                _, kind, cb, cti = combine_sched[ci]
                if kind == 'z':
                    emit_combine_z(cb, cti)
                else:
                    emit_combine_s(cb, cti)
                ci += 1
            while oi < len(out_sched) and out_sched[oi][0] <= r + 1:
                emit_out(out_sched[oi][1], out_sched[oi][2])
                oi += 1
        assert gi == len(gather_sched) and ci == len(combine_sched) \
            and oi == len(out_sched), (gi, ci, oi)


# ---------------------------------------------------------------------------
# host side
# ---------------------------------------------------------------------------

def _host_inputs(feats_a, feats_b, W, bias, coords_a, coords_b):
    """Host-side prep: fp16 casts, chunk transposes, lifted packed coords."""
    d = W.shape[1]
    dt = d // P
    bsz, na_, _ = feats_a.shape
    nb_ = feats_b.shape[1]

    ca = coords_a.astype(np.int64)
    cb = coords_b.astype(np.int64)
    a2 = (ca * ca).sum(-1)                      # [B, Na] ints < 48388
    b2 = (cb * cb).sum(-1)
    hiA, loA = a2 >> 11, a2 & 2047
    hiB, loB = b2 >> 11, b2 & 2047
    m_over = (np.arange(nb_, dtype=np.float32) / 2048.0)[None, :].repeat(bsz, 0)
    # packed dot = 2a.b - |a|^2 - |b|^2 - m/2048 = -(dist2 + m/2048)
    phia8 = np.stack([ca[..., 0], ca[..., 1], ca[..., 2], hiA, loA,
                      2048 * np.ones((bsz, na_), np.int64),
                      np.ones((bsz, na_), np.int64),
                      np.ones((bsz, na_), np.int64)], axis=1).astype(np.float16)
    phib8 = np.stack([2.0 * cb[..., 0], 2.0 * cb[..., 1], 2.0 * cb[..., 2],
                      -2048 * np.ones((bsz, nb_), np.float64),
                      -np.ones((bsz, nb_), np.float64),
                      -hiB.astype(np.float64), -loB.astype(np.float64),
                      -m_over.astype(np.float64)], axis=1).astype(np.float16)

    # feats chunk-transposed: [B, dt, 128, N]
    fatT = np.ascontiguousarray(
        feats_a.reshape(bsz, na_, dt, P).transpose(0, 2, 3, 1)).astype(np.float16)
    fbtT = np.ascontiguousarray(
        feats_b.reshape(bsz, nb_, dt, P).transpose(0, 2, 3, 1)).astype(np.float16)

    w2f = W[d:]                                  # applies to a_f
    wdf = W[:d] - W[d:]                          # applies to b_f
    w2c = np.ascontiguousarray(w2f.reshape(dt, P, d)).astype(np.float16)
    wdc = np.ascontiguousarray(wdf.reshape(dt, P, d)).astype(np.float16)
    biasw = bias.reshape(1, d).astype(np.float16)
    return fatT, fbtT, phia8, phib8, w2c, wdc, biasw


def kernel(**inputs):
    feats_a = np.asarray(inputs["feats_a"], dtype=np.float32)
    feats_b = np.asarray(inputs["feats_b"], dtype=np.float32)
    W = np.asarray(inputs["W"], dtype=np.float32)
    bias = np.asarray(inputs["bias"], dtype=np.float32)
    coords_a = np.asarray(inputs["coords_a"])
    coords_b = np.asarray(inputs["coords_b"])

    fatT, fbtT, phia8, phib8, w2c, wdc, biasw = _host_inputs(
        feats_a, feats_b, W, bias, coords_a, coords_b)
    with_bias = bool(np.any(bias != 0.0))
    identm = np.eye(P, dtype=np.float16)
    repm = np.zeros((16, P), np.float16)
    for p_ in range(P):
        repm[p_ % 16, p_] = 1.0

    nc = build_bass(with_bias=with_bias)

    in_maps = []
    for c in range(N_CORES):
        s = slice(c * BLOC, (c + 1) * BLOC)
        in_maps.append({
            "fatT": np.ascontiguousarray(fatT[s]),
            "fbtT": np.ascontiguousarray(fbtT[s]),
            "phia": np.ascontiguousarray(phia8[s]),
            "phib": np.ascontiguousarray(phib8[s]),
            "w2": w2c,
            "wd": wdc,
            "biasw": biasw,
            "ident": identm,
            "rep": repm,
        })

    from concourse import bass_utils
    res = bass_utils.run_bass_kernel_spmd(nc, in_maps, core_ids=list(range(N_CORES)))
    fused = np.concatenate([r["out"] for r in res.results], axis=0)
    return np.concatenate([feats_a, fused.astype(np.float32)], axis=-1)


if __name__ == "__main__":
    nc = build_bass()
    print("built ok")
